# revision 1
# baseline (speedup 1.0000x reference)
"""MultiHeadSelfAttention2D Trainium2 kernel (8-core SPMD).

Sharding: core c -> (batch b = c//4, head h = c%4).
Each core: QKV 1x1-conv projections + PReLU + channel-LN for its head,
full attention over T (flash-style, no max-subtraction -- LN-bounded
scores), then an AllToAll among the 4 cores of the same batch exchanges
per-head attention outputs so each core computes the final concat
projection + PReLU + LN + residual for a T/4 time-shard.

All shapes hardcoded for the problem instance:
  x [2, 64, 3000, 65], H=4 heads, D=4 q/k chans, E=16 v chans.
"""

import numpy as np
import ml_dtypes

import concourse.bass as bass
import concourse.mybir as mybir
import concourse.tile as tile
from concourse import bacc
from concourse.bass_utils import run_bass_kernel_spmd

BF16 = ml_dtypes.bfloat16

B, C, T, F = 2, 64, 3000, 65
H, D, E = 4, 4, 16
TP = 3072                    # padded T (24 tiles of 128)
TFP = TP * F                 # 199680 padded (t,f) positions
DF = D * F                   # 260  q/k embedding
EF = E * F                   # 1040 v embedding
SH = TP // 8                 # 384  t-shard per core per batch (final stage)
SHF = SH * F                 # 24960
SCALE = float(1.0 / np.sqrt(np.float32(DF)))
EPS = 1e-5

f32 = mybir.dt.float32
bf16 = mybir.dt.bfloat16

# projection tiling: each tile covers 24 consecutive t (4 col-group copies
# of 6 t each), free size 390 = 6*65
PJ_T = 6                  # t per copy
PJ_N = PJ_T * F           # 390 free
PJ_TILES = TP // (4 * PJ_T)   # 128

NQT = TP // 128           # 24 q tiles
NSB = TP // 512           # 6 s blocks of 512
S_REAL_LAST = T - 5 * 512  # 440 real cols in s-block 5


def _build_program(nrep=1, phases="123A5"):
    nc = bacc.Bacc("TRN2", target_bir_lowering=False, debug=False,
                   num_devices=8)

    def din(name, shape, dt=f32):
        return nc.dram_tensor(name, list(shape), dt, kind="ExternalInput")

    x_pad = din("x_pad", [C, TFP])
    x_res = din("x_res", [2 * C, SHF])
    w4 = din("w4", [C, 128])
    bias_v = din("bias_v", [120, 1])
    nbias_v = din("nbias_v", [120, 1])
    na_v = din("na_v", [120, 1])
    gam_v = din("gam_v", [120, 1])
    bet_v = din("bet_v", [120, 1])
    Gm = din("Gm", [120, 12])
    Bb = din("Bb", [12, 120])
    wpT = din("wpT", [C, C], bf16)
    ones64 = din("ones64", [C, C])
    nap_v = din("nap_v", [C, 1])
    bp_v = din("bp_v", [C, 1])
    nbp_v = din("nbp_v", [C, 1])
    gp_v = din("gp_v", [C, 1])
    betp_v = din("betp_v", [C, 1])
    ident_in = din("ident", [128, 128], bf16)

    y_out = nc.dram_tensor("y_shard", [2 * C, SHF], f32, kind="ExternalOutput")

    env = locals()
    with tile.TileContext(nc) as tc:
        for _rep in range(nrep):
            _body(tc, env, phases)
    nc.compile()
    return nc


def _body(tc, t, phases="123A5"):
    nc = tc.nc
    AP = bass.AP

    with tc.tile_pool(name="consts", bufs=1) as consts, \
         tc.tile_pool(name="dram", bufs=1, space="DRAM") as dram:

        # ---- constants into SBUF ----
        w4_sb = consts.tile([C, 128], f32)
        nc.sync.dma_start(w4_sb[:], t["w4"][:])
        g_sb = consts.tile([120, 12], f32)
        nc.sync.dma_start(g_sb[:], t["Gm"][:])
        bb_sb = consts.tile([12, 120], f32)
        nc.sync.dma_start(bb_sb[:], t["Bb"][:])
        vecs = {}
        for nm in ("bias_v", "nbias_v", "na_v", "gam_v", "bet_v"):
            v = consts.tile([120, 1], f32, name=nm + "_sb")
            nc.sync.dma_start(v[:], t[nm][:])
            vecs[nm] = v
        fvecs = {}
        for nm in ("nap_v", "bp_v", "nbp_v", "gp_v", "betp_v"):
            v = consts.tile([C, 1], f32, name=nm + "_sb")
            nc.sync.dma_start(v[:], t[nm][:])
            fvecs[nm] = v
        wpT_sb = consts.tile([C, C], bf16)
        nc.sync.dma_start(wpT_sb[:], t["wpT"][:])
        ones_sb = consts.tile([C, C], f32)
        nc.sync.dma_start(ones_sb[:], t["ones64"][:])
        ident_sb = consts.tile([128, 128], bf16)
        nc.sync.dma_start(ident_sb[:], t["ident_in"][:])
        eps128 = consts.tile([128, 1], f32)
        nc.vector.memset(eps128[:], EPS)

        # ---- intermediate DRAM ----
        qkv2d = dram.tile([TP, 24 * F], bf16)   # [t, (ch, f)] ch: q0-3 k0-3 v0-15
        oint = dram.tile([8, 16 * SHF], bf16)
        oall = dram.tile([128, SHF], bf16)

        x_pad = t["x_pad"]

        # ================= phase 1: QKV proj + PReLU + LN =================
        if "1" not in phases:
            return
        with tc.tile_pool(name="p1x", bufs=3) as p1x, \
             tc.tile_pool(name="p1w", bufs=3) as p1w, \
             tc.tile_pool(name="p1s", bufs=2) as p1s, \
             tc.tile_pool(name="p1ps", bufs=2, space="PSUM") as p1ps, \
             tc.tile_pool(name="p1ps1", bufs=1, space="PSUM") as p1ps1:
            for i in range(PJ_TILES):
                t0 = i * 4 * PJ_T        # first t of tile
                c0 = t0 * F              # x column offset
                x_tile = p1x.tile([C, 4 * PJ_N], f32, tag="x")
                nc.sync.dma_start(x_tile[:], x_pad[:, c0:c0 + 4 * PJ_N])

                ypsum = p1ps.tile([128, 512], f32, tag="ypsum")
                for j in range(4):
                    nc.tensor.matmul(
                        ypsum[32 * j:32 * j + 32, 0:PJ_N],
                        w4_sb[:, 32 * j:32 * j + 32],
                        x_tile[:, j * PJ_N:(j + 1) * PJ_N],
                        start=True, stop=True,
                        tile_position=(0, 32 * j),
                    )
                yp = ypsum[0:120, 0:PJ_N]

                r1 = p1w.tile([120, PJ_N], f32, tag="r1")
                nc.scalar.activation(r1[:], yp, mybir.ActivationFunctionType.Relu,
                                     bias=vecs["bias_v"][:], scale=1.0)
                r2 = p1w.tile([120, PJ_N], f32, tag="r2")
                nc.scalar.activation(r2[:], yp, mybir.ActivationFunctionType.Relu,
                                     bias=vecs["nbias_v"][:], scale=-1.0)
                r2a = p1w.tile([120, PJ_N], f32, tag="r2a")
                nc.vector.tensor_scalar(r2a[:], r2[:], vecs["na_v"][:], None,
                                        mybir.AluOpType.mult)
                y_sb = p1w.tile([120, PJ_N], f32, tag="y_sb")
                nc.vector.tensor_tensor(y_sb[:], r1[:], r2a[:],
                                        mybir.AluOpType.add)
                y2 = p1w.tile([120, PJ_N], f32, tag="y2")
                nc.scalar.activation(y2[:], y_sb[:],
                                     mybir.ActivationFunctionType.Square)

                mu_psf = p1ps.tile([12, 512], f32, tag="mu_ps")
                mu_ps = mu_psf[:, 0:PJ_N]
                nc.tensor.matmul(mu_ps, g_sb[:], y_sb[:], start=True, stop=True)
                m2_psf = p1ps.tile([12, 512], f32, tag="m2_ps")
                m2_ps = m2_psf[:, 0:PJ_N]
                nc.tensor.matmul(m2_ps, g_sb[:], y2[:], start=True, stop=True)

                spair = p1s.tile([12, 2 * PJ_N], f32, tag="spair")
                nc.scalar.copy(spair[:, 0:PJ_N], mu_ps)
                musq = p1s.tile([12, PJ_N], f32, tag="musq")
                nc.vector.tensor_tensor(musq[:], spair[:, 0:PJ_N],
                                        spair[:, 0:PJ_N], mybir.AluOpType.mult)
                var = p1s.tile([12, PJ_N], f32, tag="var")
                nc.vector.tensor_tensor(var[:], m2_ps, musq[:],
                                        mybir.AluOpType.subtract)
                stdd = p1s.tile([12, PJ_N], f32, tag="stdd")
                nc.scalar.activation(stdd[:], var[:],
                                     mybir.ActivationFunctionType.Sqrt,
                                     bias=eps128[0:12, :])
                nc.vector.reciprocal(spair[:, PJ_N:2 * PJ_N], stdd[:])

                mub = p1ps1.tile([128, 512], f32, tag="mub")
                nc.tensor.matmul(mub[0:120, 0:PJ_N], bb_sb[:], spair[:, 0:PJ_N],
                                 start=True, stop=True)
                rsb = p1ps1.tile([128, 512], f32, tag="rsb")
                nc.tensor.matmul(rsb[0:120, 0:PJ_N], bb_sb[:], spair[:, PJ_N:2 * PJ_N],
                                 start=True, stop=True)

                t1 = p1w.tile([120, PJ_N], f32, tag="t1")
                nc.vector.tensor_tensor(t1[:], y_sb[:], mub[0:120, 0:PJ_N],
                                        mybir.AluOpType.subtract)
                t2 = p1w.tile([120, PJ_N], f32, tag="t2")
                nc.vector.tensor_tensor(t2[:], t1[:], rsb[0:120, 0:PJ_N],
                                        mybir.AluOpType.mult)
                yf = p1w.tile([120, PJ_N], bf16, tag="yf")
                nc.vector.tensor_scalar(yf[:], t2[:], vecs["gam_v"][:],
                                        vecs["bet_v"][:],
                                        mybir.AluOpType.mult,
                                        mybir.AluOpType.add)

                # scatter to DRAM [t, (ch, f)]: one DMA per copy
                for j in range(4):
                    tj = t0 + j * PJ_T
                    dst = AP(tensor=qkv2d.tensor, offset=tj * 24 * F,
                             ap=[[F, 24], [24 * F, PJ_T], [1, F]])
                    nc.sync.dma_start(dst, yf[32 * j:32 * j + 24, :])

        # ================= phase 2: load K/Q emb (transpose) + V ==========
        if "2" not in phases:
            return
        with tc.tile_pool(name="attp", bufs=1) as attp:
            k_eT = []
            q_eT = []
            for ce, (e0, w) in enumerate(((0, 128), (128, 128), (256, 4))):
                kt = attp.tile([128, TP], bf16, name=f"k_eT{ce}")
                qt_ = attp.tile([128, TP], bf16, name=f"q_eT{ce}")
                for sb in range(NSB):
                    nc.sync.dma_start_transpose(
                        kt[0:w, sb * 512:(sb + 1) * 512],
                        qkv2d[sb * 512:(sb + 1) * 512, DF + e0:DF + e0 + w])
                    nc.sync.dma_start_transpose(
                        qt_[0:w, sb * 512:(sb + 1) * 512],
                        qkv2d[sb * 512:(sb + 1) * 512, e0:e0 + w])
                k_eT.append(kt)
                q_eT.append(qt_)

            v_sb = []
            for st in range(NQT):
                vt = attp.tile([128, EF], bf16, name=f"v_sb{st}")
                nc.sync.dma_start(
                    vt[:], qkv2d[st * 128:(st + 1) * 128, 2 * DF:24 * F])
                v_sb.append(vt)

            # ============== phase 3: attention ==============
            if "3" not in phases:
                return
            with tc.tile_pool(name="a3", bufs=2) as a3, \
                 tc.tile_pool(name="a3p", bufs=7) as a3p, \
                 tc.tile_pool(name="a3ps", bufs=2, space="PSUM") as a3ps, \
                 tc.tile_pool(name="a3po", bufs=1, space="PSUM") as a3po:
                for qt in range(NQT):
                    qs = slice(qt * 128, (qt + 1) * 128)
                    pblk = []
                    acc6 = a3.tile([128, 8], f32, tag="acc6")
                    for sb in range(NSB):
                        s_ps = a3ps.tile([128, 512], f32, tag="s_ps")
                        for ce, w in ((0, 128), (1, 128), (2, 4)):
                            nc.tensor.matmul(
                                s_ps[:], q_eT[ce][0:w, qs],
                                k_eT[ce][0:w, sb * 512:(sb + 1) * 512],
                                start=(ce == 0), stop=(ce == 2))
                        pb = a3p.tile([128, 512], bf16, tag=f"pb{sb}")
                        ncols = 512 if sb < NSB - 1 else S_REAL_LAST
                        nc.scalar.activation(
                            pb[:, 0:ncols], s_ps[:, 0:ncols],
                            mybir.ActivationFunctionType.Exp,
                            scale=SCALE, accum_out=acc6[:, sb:sb + 1])
                        if ncols < 512:
                            nc.vector.memset(pb[:, ncols:512], 0.0)
                        pblk.append(pb)

                    dsum = a3.tile([128, 1], f32, tag="dsum")
                    nc.vector.reduce_sum(dsum[:], acc6[:, 0:NSB],
                                         axis=mybir.AxisListType.X)
                    rcp = a3.tile([128, 1], f32, tag="rcp")
                    nc.vector.reciprocal(rcp[:], dsum[:])

                    o_ps = a3po.tile([128, 1536], f32, tag="o_ps")
                    for sb in range(NSB):
                        for c4 in range(4):
                            st = 4 * sb + c4
                            pt_ps = a3ps.tile([128, 1024], bf16, tag="pt_ps")
                            nc.tensor.transpose(
                                pt_ps[:, 0:128],
                                pblk[sb][:, c4 * 128:(c4 + 1) * 128],
                                ident_sb[:])
                            pt_sb = a3.tile([128, 128], bf16, tag="pt_sb")
                            nc.vector.tensor_copy(pt_sb[:], pt_ps[:, 0:128])
                            first, last = (st == 0), (st == NQT - 1)
                            nc.tensor.matmul(o_ps[:, 0:512], pt_sb[:],
                                             v_sb[st][:, 0:512],
                                             start=first, stop=last)
                            nc.tensor.matmul(o_ps[:, 512:1024], pt_sb[:],
                                             v_sb[st][:, 512:1024],
                                             start=first, stop=last)
                            nc.tensor.matmul(o_ps[:, 1024:EF], pt_sb[:],
                                             v_sb[st][:, 1024:EF],
                                             start=first, stop=last)

                    o_sb = a3.tile([128, EF], bf16, tag="o_sb")
                    nc.vector.tensor_scalar(o_sb[:], o_ps[:, 0:EF], rcp[:], None,
                                            mybir.AluOpType.mult)
                    sh, tl0 = qt // 3, (qt % 3) * 128
                    dst = AP(tensor=oint.tensor,
                             offset=sh * 16 * SHF + tl0 * F,
                             ap=[[F, 128], [SHF, E], [1, F]])
                    nc.sync.dma_start(dst, o_sb[:])

        # ================= phase 4: AllToAll =================
        if "A" not in phases:
            return
        nc.gpsimd.collective_compute(
            "AllToAll", mybir.AluOpType.bypass,
            replica_groups=[[0, 1, 2, 3, 4, 5, 6, 7]],
            ins=[oint[:]],
            outs=[oall.rearrange("(a c) n -> a (c n)", a=8)],
        )

        # ================= phase 5: final proj + LN + residual ============
        if "5" not in phases:
            return
        x_res = t["x_res"]
        y_out = t["y_out"]
        with tc.tile_pool(name="p5", bufs=3) as p5, \
             tc.tile_pool(name="p5ps", bufs=2, space="PSUM") as p5ps:
          nchunks = (SHF + 511) // 512
          for half in range(2):
            for k in range(nchunks):
                n0 = k * 512
                n = min(512, SHF - n0)
                o_c = p5.tile([C, 512], bf16, tag="o_c")
                nc.sync.dma_start(o_c[:, 0:n],
                                  oall[64 * half:64 * half + 64, n0:n0 + n])
                x_c = p5.tile([C, 512], f32, tag="x_c")
                nc.sync.dma_start(x_c[:, 0:n],
                                  x_res[64 * half:64 * half + 64, n0:n0 + n])

                y1 = p5ps.tile([C, 512], f32, tag="y1")
                nc.tensor.matmul(y1[:, 0:n], wpT_sb[:], o_c[:, 0:n],
                                 start=True, stop=True)
                r1 = p5.tile([C, 512], f32, tag="fr1")
                nc.scalar.activation(r1[:, 0:n], y1[:, 0:n],
                                     mybir.ActivationFunctionType.Relu,
                                     bias=fvecs["bp_v"][:], scale=1.0)
                r2 = p5.tile([C, 512], f32, tag="fr2")
                nc.scalar.activation(r2[:, 0:n], y1[:, 0:n],
                                     mybir.ActivationFunctionType.Relu,
                                     bias=fvecs["nbp_v"][:], scale=-1.0)
                r2a = p5.tile([C, 512], f32, tag="fr2a")
                nc.vector.tensor_scalar(r2a[:, 0:n], r2[:, 0:n],
                                        fvecs["nap_v"][:], None,
                                        mybir.AluOpType.mult)
                s_sb = p5.tile([C, 512], f32, tag="fs")
                nc.gpsimd.tensor_tensor(s_sb[:, 0:n], r1[:, 0:n], r2a[:, 0:n],
                                        mybir.AluOpType.add)

                mu = p5ps.tile([C, 512], f32, tag="fmu")
                nc.tensor.matmul(mu[:, 0:n], ones_sb[:], s_sb[:, 0:n],
                                 start=True, stop=True)
                t1 = p5.tile([C, 512], f32, tag="ft1")
                nc.vector.tensor_tensor(t1[:, 0:n], s_sb[:, 0:n], mu[:, 0:n],
                                        mybir.AluOpType.subtract)
                sq = p5.tile([C, 512], f32, tag="fsq")
                nc.scalar.activation(sq[:, 0:n], t1[:, 0:n],
                                     mybir.ActivationFunctionType.Square)
                vv = p5ps.tile([C, 512], f32, tag="fvar")
                nc.tensor.matmul(vv[:, 0:n], ones_sb[:], sq[:, 0:n],
                                 start=True, stop=True)
                stdd = p5.tile([C, 512], f32, tag="fstd")
                nc.scalar.activation(stdd[:, 0:n], vv[:, 0:n],
                                     mybir.ActivationFunctionType.Sqrt,
                                     bias=eps128[0:C, :])
                rstd = p5.tile([C, 512], f32, tag="frstd")
                nc.vector.reciprocal(rstd[:, 0:n], stdd[:, 0:n])
                yn = p5.tile([C, 512], f32, tag="fyn")
                nc.vector.tensor_tensor(yn[:, 0:n], t1[:, 0:n], rstd[:, 0:n],
                                        mybir.AluOpType.mult)
                yg = p5.tile([C, 512], f32, tag="fyg")
                nc.vector.tensor_scalar(yg[:, 0:n], yn[:, 0:n],
                                        fvecs["gp_v"][:], fvecs["betp_v"][:],
                                        mybir.AluOpType.mult,
                                        mybir.AluOpType.add)
                yo = p5.tile([C, 512], f32, tag="fyo")
                nc.gpsimd.tensor_tensor(yo[:, 0:n], yg[:, 0:n], x_c[:, 0:n],
                                        mybir.AluOpType.add)
                nc.sync.dma_start(y_out[64 * half:64 * half + 64, n0:n0 + n],
                                  yo[:, 0:n])


_PROGRAM = None


def _get_program():
    global _PROGRAM
    if _PROGRAM is None:
        _PROGRAM = _build_program()
    return _PROGRAM


def _core_inputs(inp, c):
    b, h = c // 4, c % 4
    x = np.asarray(inp["x"], np.float32)
    xb = np.zeros((B, C, TP, F), np.float32)
    xb[:, :, :T, :] = x
    x_pad = np.ascontiguousarray(xb[b].reshape(C, TFP))
    # final-stage residual: eighth-shard c of BOTH batches, stacked [2C, SHF]
    xs = xb[:, :, SH * c:SH * (c + 1), :].reshape(B * C, SHF)
    x_res = np.ascontiguousarray(xs)

    Wq, Wk, Wv = (np.asarray(inp[k], np.float32) for k in ("Wq", "Wk", "Wv"))
    bq, bk, bv = (np.asarray(inp[k], np.float32) for k in ("bq", "bk", "bv"))
    aq, ak, av = (np.asarray(inp[k], np.float32) for k in ("aq", "ak", "av"))
    gq, gk, gv = (np.asarray(inp[k], np.float32) for k in ("gq", "gk", "gv"))
    btq, btk, btv = (np.asarray(inp[k], np.float32)
                     for k in ("betaq", "betak", "betav"))

    w24 = np.concatenate([Wq[h], Wk[h], Wv[h]], axis=0)     # [24, C]
    b24 = np.concatenate([bq[h], bk[h], bv[h]])             # [24]
    a24 = np.concatenate([np.full(D, aq[h]), np.full(D, ak[h]),
                          np.full(E, av[h])]).astype(np.float32)
    g24 = np.concatenate([gq[h], gk[h], gv[h]])
    bt24 = np.concatenate([btq[h], btk[h], btv[h]])

    w4 = np.zeros((C, 128), np.float32)
    bias_v = np.zeros((120, 1), np.float32)
    na_v = np.zeros((120, 1), np.float32)
    gam_v = np.zeros((120, 1), np.float32)
    bet_v = np.zeros((120, 1), np.float32)
    G = np.zeros((120, 12), np.float32)
    Bbm = np.zeros((12, 120), np.float32)
    for j in range(4):
        r = 32 * j
        w4[:, r:r + 24] = w24.T
        bias_v[r:r + 24, 0] = b24
        na_v[r:r + 24, 0] = -a24
        gam_v[r:r + 24, 0] = g24
        bet_v[r:r + 24, 0] = bt24
        G[r:r + 4, 3 * j + 0] = 0.25
        G[r + 4:r + 8, 3 * j + 1] = 0.25
        G[r + 8:r + 24, 3 * j + 2] = 1.0 / 16.0
        Bbm[3 * j + 0, r:r + 4] = 1.0
        Bbm[3 * j + 1, r + 4:r + 8] = 1.0
        Bbm[3 * j + 2, r + 8:r + 24] = 1.0

    Wp = np.asarray(inp["Wp"], np.float32)
    bp = np.asarray(inp["bp"], np.float32)
    ap = np.float32(inp["ap"])
    gp = np.asarray(inp["gp"], np.float32)
    betp = np.asarray(inp["betap"], np.float32)

    return {
        "x_pad": x_pad,
        "x_res": x_res,
        "w4": w4,
        "bias_v": bias_v,
        "nbias_v": -bias_v,
        "na_v": na_v,
        "gam_v": gam_v,
        "bet_v": bet_v,
        "Gm": G,
        "Bb": Bbm,
        "wpT": np.ascontiguousarray(Wp.T).astype(BF16),
        "ones64": np.full((C, C), 1.0 / 64.0, np.float32),
        "nap_v": np.full((C, 1), -ap, np.float32),
        "bp_v": bp.reshape(C, 1).copy(),
        "nbp_v": (-bp).reshape(C, 1).copy(),
        "gp_v": gp.reshape(C, 1).copy(),
        "betp_v": betp.reshape(C, 1).copy(),
        "ident": np.eye(128, dtype=BF16),
    }


def gather_output(results):
    y = np.empty((B, C, T, F), np.float32)
    for c in range(8):
        sh = np.asarray(results[c]["y_shard"], np.float32).reshape(B, C, SH, F)
        t0, t1 = SH * c, min(SH * (c + 1), T)
        if t1 > t0:
            y[:, :, t0:t1, :] = sh[:, :, :t1 - t0, :]
    return y


def kernel(**inputs):
    nc = _get_program()
    in_maps = [_core_inputs(inputs, c) for c in range(8)]
    res = run_bass_kernel_spmd(nc, in_maps, core_ids=list(range(8)))
    return gather_output(res.results)



# revision 2
# speedup vs baseline: 5.4792x; 5.4792x over previous
"""MultiHeadSelfAttention2D Trainium2 kernel (8-core SPMD).

v2: input-minimal T-sharded design.

Each core receives ONLY its 1/8 time-shard of x (both batches, bf16,
~6.4 MB) plus small weight tensors.  Flow per core:

  phase 1: QKV 1x1-conv + PReLU + channel-LN for ALL 4 heads and BOTH
           batches on the core's 384-t shard (96 output channels).
  phase C: AllToAll #1 redistributes QKV t-shards -> (batch, head)
           shards; core d = 4*b + h ends with full-T qkv2d [3072, 1560]
           in [t, (ch, f)] layout for its (b, h).
  phase 2: load K/Q embeddings (DMA transpose) + V into SBUF.
  phase 3: full attention over T (exp without max-subtraction --
           LN-bounded scores), P^T V accumulation.
  phase A: AllToAll #2 exchanges per-head attention outputs so each
           core holds all 16 v-chan groups for a 384-t shard of both
           batches.
  phase 5: final concat 1x1-conv + PReLU + channel-LN + residual on the
           same t-shard (residual re-reads x_sh).

All shapes hardcoded for the problem instance:
  x [2, 64, 3000, 65], H=4 heads, D=4 q/k chans, E=16 v chans.
"""

import numpy as np
import ml_dtypes

import concourse.bass as bass
import concourse.mybir as mybir
import concourse.tile as tile
from concourse import bacc
from concourse.bass_utils import run_bass_kernel_spmd

BF16 = ml_dtypes.bfloat16

B, C, T, F = 2, 64, 3000, 65
H, D, E = 4, 4, 16
TP = 3072                    # padded T (24 tiles of 128)
DF = D * F                   # 260  q/k embedding
EF = E * F                   # 1040 v embedding
SH = TP // 8                 # 384  t-shard per core per batch
SHF = SH * F                 # 24960
ROWW = 24 * F                # 1560 qkv2d row width: 24 chans x 65 f
SCALE = float(1.0 / np.sqrt(np.float32(DF)))
EPS = 1e-5

f32 = mybir.dt.float32
bf16 = mybir.dt.bfloat16

# phase-1 tiling: chunk of 6 t (390 (t,f) positions) per iteration
PJ_T = 6
PJ_N = PJ_T * F              # 390 free
PJ_TILES = SH // PJ_T        # 64 chunks per batch

NQT = TP // 128              # 24 q tiles
NSB = TP // 512              # 6 s blocks of 512
S_REAL_LAST = T - 5 * 512    # 440 real cols in s-block 5


def _build_program(nrep=1, phases="1C23A5"):
    nc = bacc.Bacc("TRN2", target_bir_lowering=False, debug=False,
                   num_devices=8)

    def din(name, shape, dt=f32):
        return nc.dram_tensor(name, list(shape), dt, kind="ExternalInput")

    x_sh = din("x_sh", [2 * C, SHF], bf16)
    w96 = din("w96", [C, 96], bf16)
    bias_v = din("bias_v", [96, 1])
    nbias_v = din("nbias_v", [96, 1])
    na_v = din("na_v", [96, 1])
    gam_v = din("gam_v", [96, 1])
    bet_v = din("bet_v", [96, 1])
    Gm = din("Gm", [96, 12])
    Bb = din("Bb", [12, 96])
    wpT = din("wpT", [C, C], bf16)
    ones64 = din("ones64", [C, C])
    nap_v = din("nap_v", [C, 1])
    bp_v = din("bp_v", [C, 1])
    nbp_v = din("nbp_v", [C, 1])
    gp_v = din("gp_v", [C, 1])
    betp_v = din("betp_v", [C, 1])
    ident_in = din("ident", [128, 128], bf16)

    y_out = nc.dram_tensor("y_shard", [2 * C, SHF], f32, kind="ExternalOutput")

    env = locals()
    with tile.TileContext(nc) as tc:
        for _rep in range(nrep):
            _body(tc, env, phases)
    nc.compile()
    return nc


def _body(tc, t, phases="1C23A5"):
    nc = tc.nc
    AP = bass.AP

    with tc.tile_pool(name="consts", bufs=1) as consts, \
         tc.tile_pool(name="dram", bufs=1, space="DRAM") as dram:

        # ---- constants into SBUF ----
        w96_sb = consts.tile([C, 96], bf16)
        nc.sync.dma_start(w96_sb[:], t["w96"][:])
        g_sb = consts.tile([96, 12], f32)
        nc.sync.dma_start(g_sb[:], t["Gm"][:])
        bb_sb = consts.tile([12, 96], f32)
        nc.sync.dma_start(bb_sb[:], t["Bb"][:])
        vecs = {}
        for nm in ("bias_v", "nbias_v", "na_v", "gam_v", "bet_v"):
            v = consts.tile([96, 1], f32, name=nm + "_sb")
            nc.sync.dma_start(v[:], t[nm][:])
            vecs[nm] = v
        fvecs = {}
        for nm in ("nap_v", "bp_v", "nbp_v", "gp_v", "betp_v"):
            v = consts.tile([C, 1], f32, name=nm + "_sb")
            nc.sync.dma_start(v[:], t[nm][:])
            fvecs[nm] = v
        wpT_sb = consts.tile([C, C], bf16)
        nc.sync.dma_start(wpT_sb[:], t["wpT"][:])
        ones_sb = consts.tile([C, C], f32)
        nc.sync.dma_start(ones_sb[:], t["ones64"][:])
        ident_sb = consts.tile([128, 128], bf16)
        nc.sync.dma_start(ident_sb[:], t["ident_in"][:])
        eps128 = consts.tile([128, 1], f32)
        nc.vector.memset(eps128[:], EPS)

        # ---- intermediate DRAM ----
        qkvsend = dram.tile([8, SH * ROWW], bf16)   # dest-major AllToAll stage
        qkv2d = dram.tile([TP, ROWW], bf16)         # [t, (ch, f)] q0-3 k0-3 v0-15
        oint = dram.tile([8, 16 * SHF], bf16)
        oall = dram.tile([128, SHF], bf16)

        x_sh = t["x_sh"]

        # ======== phase 1: QKV proj + PReLU + LN (all heads, both b) ======
        if "1" in phases:
            with tc.tile_pool(name="p1x", bufs=1) as p1x, \
                 tc.tile_pool(name="p1w", bufs=3) as p1w, \
                 tc.tile_pool(name="p1s", bufs=2) as p1s, \
                 tc.tile_pool(name="p1ps", bufs=2, space="PSUM") as p1ps, \
                 tc.tile_pool(name="p1ps1", bufs=1, space="PSUM") as p1ps1:
                xb = []
                for b in range(2):
                    xt = p1x.tile([C, SHF], bf16, name=f"xb{b}")
                    nc.sync.dma_start(xt[:], x_sh[64 * b:64 * b + 64, :])
                    xb.append(xt)
                for it in range(2 * PJ_TILES):
                    b, i = it // PJ_TILES, it % PJ_TILES
                    n0 = i * PJ_N
                    ypsum = p1ps.tile([128, 512], f32, tag="ypsum")
                    nc.tensor.matmul(ypsum[0:96, 0:PJ_N], w96_sb[:],
                                     xb[b][:, n0:n0 + PJ_N],
                                     start=True, stop=True)
                    yp = ypsum[0:96, 0:PJ_N]

                    r1 = p1w.tile([96, PJ_N], f32, tag="r1")
                    nc.scalar.activation(r1[:], yp,
                                         mybir.ActivationFunctionType.Relu,
                                         bias=vecs["bias_v"][:], scale=1.0)
                    r2 = p1w.tile([96, PJ_N], f32, tag="r2")
                    nc.scalar.activation(r2[:], yp,
                                         mybir.ActivationFunctionType.Relu,
                                         bias=vecs["nbias_v"][:], scale=-1.0)
                    r2a = p1w.tile([96, PJ_N], f32, tag="r2a")
                    nc.vector.tensor_scalar(r2a[:], r2[:], vecs["na_v"][:],
                                            None, mybir.AluOpType.mult)
                    y_sb = p1w.tile([96, PJ_N], f32, tag="y_sb")
                    nc.vector.tensor_tensor(y_sb[:], r1[:], r2a[:],
                                            mybir.AluOpType.add)
                    y2 = p1w.tile([96, PJ_N], f32, tag="y2")
                    nc.scalar.activation(y2[:], y_sb[:],
                                         mybir.ActivationFunctionType.Square)

                    mu_psf = p1ps.tile([12, 512], f32, tag="mu_ps")
                    mu_ps = mu_psf[:, 0:PJ_N]
                    nc.tensor.matmul(mu_ps, g_sb[:], y_sb[:],
                                     start=True, stop=True)
                    m2_psf = p1ps.tile([12, 512], f32, tag="m2_ps")
                    m2_ps = m2_psf[:, 0:PJ_N]
                    nc.tensor.matmul(m2_ps, g_sb[:], y2[:],
                                     start=True, stop=True)

                    spair = p1s.tile([12, 2 * PJ_N], f32, tag="spair")
                    nc.scalar.copy(spair[:, 0:PJ_N], mu_ps)
                    musq = p1s.tile([12, PJ_N], f32, tag="musq")
                    nc.vector.tensor_tensor(musq[:], spair[:, 0:PJ_N],
                                            spair[:, 0:PJ_N],
                                            mybir.AluOpType.mult)
                    var = p1s.tile([12, PJ_N], f32, tag="var")
                    nc.vector.tensor_tensor(var[:], m2_ps, musq[:],
                                            mybir.AluOpType.subtract)
                    stdd = p1s.tile([12, PJ_N], f32, tag="stdd")
                    nc.scalar.activation(stdd[:], var[:],
                                         mybir.ActivationFunctionType.Sqrt,
                                         bias=eps128[0:12, :])
                    nc.vector.reciprocal(spair[:, PJ_N:2 * PJ_N], stdd[:])

                    mub = p1ps1.tile([128, 512], f32, tag="mub")
                    nc.tensor.matmul(mub[0:96, 0:PJ_N], bb_sb[:],
                                     spair[:, 0:PJ_N], start=True, stop=True)
                    rsb = p1ps1.tile([128, 512], f32, tag="rsb")
                    nc.tensor.matmul(rsb[0:96, 0:PJ_N], bb_sb[:],
                                     spair[:, PJ_N:2 * PJ_N],
                                     start=True, stop=True)

                    t1 = p1w.tile([96, PJ_N], f32, tag="t1")
                    nc.vector.tensor_tensor(t1[:], y_sb[:], mub[0:96, 0:PJ_N],
                                            mybir.AluOpType.subtract)
                    t2 = p1w.tile([96, PJ_N], f32, tag="t2")
                    nc.vector.tensor_tensor(t2[:], t1[:], rsb[0:96, 0:PJ_N],
                                            mybir.AluOpType.mult)
                    yf = p1w.tile([96, PJ_N], bf16, tag="yf")
                    nc.vector.tensor_scalar(yf[:], t2[:], vecs["gam_v"][:],
                                            vecs["bet_v"][:],
                                            mybir.AluOpType.mult,
                                            mybir.AluOpType.add)

                    # scatter to qkvsend[4b+h, tl*1560 + ch*65 + f]
                    for h in range(4):
                        dst = AP(tensor=qkvsend.tensor,
                                 offset=(4 * b + h) * SH * ROWW
                                 + i * PJ_T * ROWW,
                                 ap=[[F, 24], [ROWW, PJ_T], [1, F]])
                        nc.sync.dma_start(dst, yf[24 * h:24 * h + 24, :])

        # ======== phase C: AllToAll #1 (t-shard -> (b,h)-shard) ==========
        if "C" in phases:
            nc.gpsimd.collective_compute(
                "AllToAll", mybir.AluOpType.bypass,
                replica_groups=[[0, 1, 2, 3, 4, 5, 6, 7]],
                ins=[qkvsend[:]],
                outs=[qkv2d.rearrange("(a t) n -> a (t n)", a=8)],
            )

        # ======== phase 2: load K/Q emb (transpose) + V ==========
        if "2" in phases:
            with tc.tile_pool(name="attp", bufs=1) as attp:
                k_eT = []
                q_eT = []
                for ce, (e0, w) in enumerate(((0, 128), (128, 128), (256, 4))):
                    kt = attp.tile([128, TP], bf16, name=f"k_eT{ce}")
                    qt_ = attp.tile([128, TP], bf16, name=f"q_eT{ce}")
                    for sb in range(NSB):
                        nc.sync.dma_start_transpose(
                            kt[0:w, sb * 512:(sb + 1) * 512],
                            qkv2d[sb * 512:(sb + 1) * 512,
                                  DF + e0:DF + e0 + w])
                        nc.sync.dma_start_transpose(
                            qt_[0:w, sb * 512:(sb + 1) * 512],
                            qkv2d[sb * 512:(sb + 1) * 512, e0:e0 + w])
                    k_eT.append(kt)
                    q_eT.append(qt_)

                v_sb = []
                for st in range(NQT):
                    vt = attp.tile([128, EF], bf16, name=f"v_sb{st}")
                    nc.sync.dma_start(
                        vt[:], qkv2d[st * 128:(st + 1) * 128, 2 * DF:ROWW])
                    v_sb.append(vt)

                # ============== phase 3: attention ==============
                if "3" in phases:
                    with tc.tile_pool(name="a3", bufs=2) as a3, \
                         tc.tile_pool(name="a3p", bufs=7) as a3p, \
                         tc.tile_pool(name="a3ps", bufs=2, space="PSUM") as a3ps, \
                         tc.tile_pool(name="a3po", bufs=1, space="PSUM") as a3po:
                        for qt in range(NQT):
                            qs = slice(qt * 128, (qt + 1) * 128)
                            pblk = []
                            acc6 = a3.tile([128, 8], f32, tag="acc6")
                            for sb in range(NSB):
                                s_ps = a3ps.tile([128, 512], f32, tag="s_ps")
                                for ce, w in ((0, 128), (1, 128), (2, 4)):
                                    nc.tensor.matmul(
                                        s_ps[:], q_eT[ce][0:w, qs],
                                        k_eT[ce][0:w, sb * 512:(sb + 1) * 512],
                                        start=(ce == 0), stop=(ce == 2))
                                pb = a3p.tile([128, 512], bf16, tag=f"pb{sb}")
                                ncols = 512 if sb < NSB - 1 else S_REAL_LAST
                                nc.scalar.activation(
                                    pb[:, 0:ncols], s_ps[:, 0:ncols],
                                    mybir.ActivationFunctionType.Exp,
                                    scale=SCALE, accum_out=acc6[:, sb:sb + 1])
                                if ncols < 512:
                                    nc.vector.memset(pb[:, ncols:512], 0.0)
                                pblk.append(pb)

                            dsum = a3.tile([128, 1], f32, tag="dsum")
                            nc.vector.reduce_sum(dsum[:], acc6[:, 0:NSB],
                                                 axis=mybir.AxisListType.X)
                            rcp = a3.tile([128, 1], f32, tag="rcp")
                            nc.vector.reciprocal(rcp[:], dsum[:])

                            o_ps = a3po.tile([128, 1536], f32, tag="o_ps")
                            for sb in range(NSB):
                                for c4 in range(4):
                                    st = 4 * sb + c4
                                    pt_ps = a3ps.tile([128, 1024], bf16,
                                                      tag="pt_ps")
                                    nc.tensor.transpose(
                                        pt_ps[:, 0:128],
                                        pblk[sb][:, c4 * 128:(c4 + 1) * 128],
                                        ident_sb[:])
                                    pt_sb = a3.tile([128, 128], bf16,
                                                    tag="pt_sb")
                                    nc.vector.tensor_copy(pt_sb[:],
                                                          pt_ps[:, 0:128])
                                    first, last = (st == 0), (st == NQT - 1)
                                    nc.tensor.matmul(o_ps[:, 0:512], pt_sb[:],
                                                     v_sb[st][:, 0:512],
                                                     start=first, stop=last)
                                    nc.tensor.matmul(o_ps[:, 512:1024],
                                                     pt_sb[:],
                                                     v_sb[st][:, 512:1024],
                                                     start=first, stop=last)
                                    nc.tensor.matmul(o_ps[:, 1024:EF],
                                                     pt_sb[:],
                                                     v_sb[st][:, 1024:EF],
                                                     start=first, stop=last)

                            o_sb = a3.tile([128, EF], bf16, tag="o_sb")
                            nc.vector.tensor_scalar(o_sb[:], o_ps[:, 0:EF],
                                                    rcp[:], None,
                                                    mybir.AluOpType.mult)
                            sh, tl0 = qt // 3, (qt % 3) * 128
                            dst = AP(tensor=oint.tensor,
                                     offset=sh * 16 * SHF + tl0 * F,
                                     ap=[[F, 128], [SHF, E], [1, F]])
                            nc.sync.dma_start(dst, o_sb[:])

        # ======== phase A: AllToAll #2 ==========
        if "A" in phases:
            nc.gpsimd.collective_compute(
                "AllToAll", mybir.AluOpType.bypass,
                replica_groups=[[0, 1, 2, 3, 4, 5, 6, 7]],
                ins=[oint[:]],
                outs=[oall.rearrange("(a c) n -> a (c n)", a=8)],
            )

        # ======== phase 5: final proj + LN + residual ==========
        if "5" in phases:
            y_out = t["y_out"]
            with tc.tile_pool(name="p5", bufs=3) as p5, \
                 tc.tile_pool(name="p5ps", bufs=2, space="PSUM") as p5ps:
              nchunks = (SHF + 511) // 512
              for half in range(2):
                for k in range(nchunks):
                    n0 = k * 512
                    n = min(512, SHF - n0)
                    o_c = p5.tile([C, 512], bf16, tag="o_c")
                    nc.sync.dma_start(o_c[:, 0:n],
                                      oall[64 * half:64 * half + 64,
                                           n0:n0 + n])
                    x_c = p5.tile([C, 512], bf16, tag="x_c")
                    nc.sync.dma_start(x_c[:, 0:n],
                                      x_sh[64 * half:64 * half + 64,
                                           n0:n0 + n])

                    y1 = p5ps.tile([C, 512], f32, tag="y1")
                    nc.tensor.matmul(y1[:, 0:n], wpT_sb[:], o_c[:, 0:n],
                                     start=True, stop=True)
                    r1 = p5.tile([C, 512], f32, tag="fr1")
                    nc.scalar.activation(r1[:, 0:n], y1[:, 0:n],
                                         mybir.ActivationFunctionType.Relu,
                                         bias=fvecs["bp_v"][:], scale=1.0)
                    r2 = p5.tile([C, 512], f32, tag="fr2")
                    nc.scalar.activation(r2[:, 0:n], y1[:, 0:n],
                                         mybir.ActivationFunctionType.Relu,
                                         bias=fvecs["nbp_v"][:], scale=-1.0)
                    r2a = p5.tile([C, 512], f32, tag="fr2a")
                    nc.vector.tensor_scalar(r2a[:, 0:n], r2[:, 0:n],
                                            fvecs["nap_v"][:], None,
                                            mybir.AluOpType.mult)
                    s_sb = p5.tile([C, 512], f32, tag="fs")
                    nc.gpsimd.tensor_tensor(s_sb[:, 0:n], r1[:, 0:n],
                                            r2a[:, 0:n], mybir.AluOpType.add)

                    mu = p5ps.tile([C, 512], f32, tag="fmu")
                    nc.tensor.matmul(mu[:, 0:n], ones_sb[:], s_sb[:, 0:n],
                                     start=True, stop=True)
                    t1 = p5.tile([C, 512], f32, tag="ft1")
                    nc.vector.tensor_tensor(t1[:, 0:n], s_sb[:, 0:n],
                                            mu[:, 0:n],
                                            mybir.AluOpType.subtract)
                    sq = p5.tile([C, 512], f32, tag="fsq")
                    nc.scalar.activation(sq[:, 0:n], t1[:, 0:n],
                                         mybir.ActivationFunctionType.Square)
                    vv = p5ps.tile([C, 512], f32, tag="fvar")
                    nc.tensor.matmul(vv[:, 0:n], ones_sb[:], sq[:, 0:n],
                                     start=True, stop=True)
                    stdd = p5.tile([C, 512], f32, tag="fstd")
                    nc.scalar.activation(stdd[:, 0:n], vv[:, 0:n],
                                         mybir.ActivationFunctionType.Sqrt,
                                         bias=eps128[0:C, :])
                    rstd = p5.tile([C, 512], f32, tag="frstd")
                    nc.vector.reciprocal(rstd[:, 0:n], stdd[:, 0:n])
                    yn = p5.tile([C, 512], f32, tag="fyn")
                    nc.vector.tensor_tensor(yn[:, 0:n], t1[:, 0:n],
                                            rstd[:, 0:n],
                                            mybir.AluOpType.mult)
                    yg = p5.tile([C, 512], f32, tag="fyg")
                    nc.vector.tensor_scalar(yg[:, 0:n], yn[:, 0:n],
                                            fvecs["gp_v"][:],
                                            fvecs["betp_v"][:],
                                            mybir.AluOpType.mult,
                                            mybir.AluOpType.add)
                    yo = p5.tile([C, 512], f32, tag="fyo")
                    nc.gpsimd.tensor_tensor(yo[:, 0:n], yg[:, 0:n],
                                            x_c[:, 0:n], mybir.AluOpType.add)
                    nc.sync.dma_start(y_out[64 * half:64 * half + 64,
                                            n0:n0 + n], yo[:, 0:n])


_PROGRAM = None


def _get_program():
    global _PROGRAM
    if _PROGRAM is None:
        _PROGRAM = _build_program()
    return _PROGRAM


def _core_inputs(inp, c):
    x = np.asarray(inp["x"], np.float32)
    xb = np.zeros((B, C, TP, F), np.float32)
    xb[:, :, :T, :] = x
    # core c's t-shard of BOTH batches, [2C, SHF], bf16
    x_sh = np.ascontiguousarray(
        xb[:, :, SH * c:SH * (c + 1), :].reshape(2 * C, SHF)).astype(BF16)

    Wq, Wk, Wv = (np.asarray(inp[k], np.float32) for k in ("Wq", "Wk", "Wv"))
    bq, bk, bv = (np.asarray(inp[k], np.float32) for k in ("bq", "bk", "bv"))
    aq, ak, av = (np.asarray(inp[k], np.float32) for k in ("aq", "ak", "av"))
    gq, gk, gv = (np.asarray(inp[k], np.float32) for k in ("gq", "gk", "gv"))
    btq, btk, btv = (np.asarray(inp[k], np.float32)
                     for k in ("betaq", "betak", "betav"))

    w96 = np.zeros((C, 96), np.float32)
    bias_v = np.zeros((96, 1), np.float32)
    na_v = np.zeros((96, 1), np.float32)
    gam_v = np.zeros((96, 1), np.float32)
    bet_v = np.zeros((96, 1), np.float32)
    G = np.zeros((96, 12), np.float32)
    Bbm = np.zeros((12, 96), np.float32)
    for h in range(4):
        r = 24 * h
        w24 = np.concatenate([Wq[h], Wk[h], Wv[h]], axis=0)     # [24, C]
        w96[:, r:r + 24] = w24.T
        bias_v[r:r + 24, 0] = np.concatenate([bq[h], bk[h], bv[h]])
        a24 = np.concatenate([np.full(D, aq[h]), np.full(D, ak[h]),
                              np.full(E, av[h])]).astype(np.float32)
        na_v[r:r + 24, 0] = -a24
        gam_v[r:r + 24, 0] = np.concatenate([gq[h], gk[h], gv[h]])
        bet_v[r:r + 24, 0] = np.concatenate([btq[h], btk[h], btv[h]])
        G[r:r + 4, 3 * h + 0] = 0.25
        G[r + 4:r + 8, 3 * h + 1] = 0.25
        G[r + 8:r + 24, 3 * h + 2] = 1.0 / 16.0
        Bbm[3 * h + 0, r:r + 4] = 1.0
        Bbm[3 * h + 1, r + 4:r + 8] = 1.0
        Bbm[3 * h + 2, r + 8:r + 24] = 1.0

    Wp = np.asarray(inp["Wp"], np.float32)
    bp = np.asarray(inp["bp"], np.float32)
    ap = np.float32(inp["ap"])
    gp = np.asarray(inp["gp"], np.float32)
    betp = np.asarray(inp["betap"], np.float32)

    return {
        "x_sh": x_sh,
        "w96": w96.astype(BF16),
        "bias_v": bias_v,
        "nbias_v": -bias_v,
        "na_v": na_v,
        "gam_v": gam_v,
        "bet_v": bet_v,
        "Gm": G,
        "Bb": Bbm,
        "wpT": np.ascontiguousarray(Wp.T).astype(BF16),
        "ones64": np.full((C, C), 1.0 / 64.0, np.float32),
        "nap_v": np.full((C, 1), -ap, np.float32),
        "bp_v": bp.reshape(C, 1).copy(),
        "nbp_v": (-bp).reshape(C, 1).copy(),
        "gp_v": gp.reshape(C, 1).copy(),
        "betp_v": betp.reshape(C, 1).copy(),
        "ident": np.eye(128, dtype=BF16),
    }


def gather_output(results):
    y = np.empty((B, C, T, F), np.float32)
    for c in range(8):
        sh = np.asarray(results[c]["y_shard"], np.float32).reshape(B, C, SH, F)
        t0, t1 = SH * c, min(SH * (c + 1), T)
        if t1 > t0:
            y[:, :, t0:t1, :] = sh[:, :, :t1 - t0, :]
    return y


def kernel(**inputs):
    nc = _get_program()
    in_maps = [_core_inputs(inputs, c) for c in range(8)]
    res = run_bass_kernel_spmd(nc, in_maps, core_ids=list(range(8)))
    return gather_output(res.results)


# revision 4
# speedup vs baseline: 7.9457x; 1.4501x over previous
"""MultiHeadSelfAttention2D Trainium2 kernel (8-core SPMD).

v2: input-minimal T-sharded design.

Each core receives ONLY its 1/8 time-shard of x (both batches, bf16,
~6.4 MB) plus small weight tensors.  Flow per core:

  phase 1: QKV 1x1-conv + PReLU + channel-LN for ALL 4 heads and BOTH
           batches on the core's 384-t shard (96 output channels).
  phase C: AllToAll #1 redistributes QKV t-shards -> (batch, head)
           shards; core d = 4*b + h ends with full-T qkv2d [3072, 1560]
           in [t, (ch, f)] layout for its (b, h).
  phase 2: load K/Q embeddings (DMA transpose) + V into SBUF.
  phase 3: full attention over T (exp without max-subtraction --
           LN-bounded scores), P^T V accumulation.
  phase A: AllToAll #2 exchanges per-head attention outputs so each
           core holds all 16 v-chan groups for a 384-t shard of both
           batches.
  phase 5: final concat 1x1-conv + PReLU + channel-LN + residual on the
           same t-shard (residual re-reads x_sh).

All shapes hardcoded for the problem instance:
  x [2, 64, 3000, 65], H=4 heads, D=4 q/k chans, E=16 v chans.
"""

import numpy as np
import ml_dtypes

import concourse.bass as bass
import concourse.mybir as mybir
import concourse.tile as tile
from concourse import bacc
from concourse.bass_utils import run_bass_kernel_spmd

BF16 = ml_dtypes.bfloat16

B, C, T, F = 2, 64, 3000, 65
H, D, E = 4, 4, 16
TP = 3072                    # padded T (24 tiles of 128)
DF = D * F                   # 260  q/k embedding
EF = E * F                   # 1040 v embedding
SH = TP // 8                 # 384  t-shard per core per batch
SHF = SH * F                 # 24960
ROWW = 24 * F                # 1560 qkv2d row width: 24 chans x 65 f
SCALE = float(1.0 / np.sqrt(np.float32(DF)))
EPS = 1e-5

f32 = mybir.dt.float32
bf16 = mybir.dt.bfloat16

# phase-1 tiling: chunk of 6 t (390 (t,f) positions) per iteration
PJ_T = 6
PJ_N = PJ_T * F              # 390 free
PJ_TILES = SH // PJ_T        # 64 chunks per batch

NQT = TP // 128              # 24 q tiles
NSB = TP // 512              # 6 s blocks of 512
S_REAL_LAST = T - 5 * 512    # 440 real cols in s-block 5


def _build_program(nrep=1, phases="1C23A5"):
    nc = bacc.Bacc("TRN2", target_bir_lowering=False, debug=False,
                   num_devices=8)

    def din(name, shape, dt=f32):
        return nc.dram_tensor(name, list(shape), dt, kind="ExternalInput")

    x_sh = din("x_sh", [2 * C, SHF], bf16)
    w96 = din("w96", [C, 96], bf16)
    bias_v = din("bias_v", [96, 1])
    nbias_v = din("nbias_v", [96, 1])
    na_v = din("na_v", [96, 1])
    gam_v = din("gam_v", [96, 1])
    bet_v = din("bet_v", [96, 1])
    Gm = din("Gm", [96, 12])
    Bb = din("Bb", [12, 96])
    wpT = din("wpT", [C, C], bf16)
    ones64 = din("ones64", [C, C])
    nap_v = din("nap_v", [C, 1])
    bp_v = din("bp_v", [C, 1])
    nbp_v = din("nbp_v", [C, 1])
    gp_v = din("gp_v", [C, 1])
    betp_v = din("betp_v", [C, 1])
    ident_in = din("ident", [128, 128], bf16)

    y_out = nc.dram_tensor("y_shard", [2 * C, SHF], bf16, kind="ExternalOutput")

    env = locals()
    with tile.TileContext(nc) as tc:
        for _rep in range(nrep):
            _body(tc, env, phases)
    nc.compile()
    return nc


def _body(tc, t, phases="1C23A5"):
    nc = tc.nc
    AP = bass.AP

    with tc.tile_pool(name="consts", bufs=1) as consts, \
         tc.tile_pool(name="dram", bufs=1, space="DRAM") as dram:

        # ---- constants into SBUF ----
        w96_sb = consts.tile([C, 96], bf16)
        nc.sync.dma_start(w96_sb[:], t["w96"][:])
        g_sb = consts.tile([96, 12], f32)
        nc.sync.dma_start(g_sb[:], t["Gm"][:])
        bb_sb = consts.tile([12, 96], f32)
        nc.sync.dma_start(bb_sb[:], t["Bb"][:])
        vecs = {}
        for nm in ("bias_v", "nbias_v", "na_v", "gam_v", "bet_v"):
            v = consts.tile([96, 1], f32, name=nm + "_sb")
            nc.sync.dma_start(v[:], t[nm][:])
            vecs[nm] = v
        fvecs = {}
        for nm in ("nap_v", "bp_v", "nbp_v", "gp_v", "betp_v"):
            v = consts.tile([C, 1], f32, name=nm + "_sb")
            nc.sync.dma_start(v[:], t[nm][:])
            fvecs[nm] = v
        wpT_sb = consts.tile([C, C], bf16)
        nc.sync.dma_start(wpT_sb[:], t["wpT"][:])
        ones_sb = consts.tile([C, C], f32)
        nc.sync.dma_start(ones_sb[:], t["ones64"][:])
        ident_sb = consts.tile([128, 128], bf16)
        nc.sync.dma_start(ident_sb[:], t["ident_in"][:])
        eps128 = consts.tile([128, 1], f32)
        nc.vector.memset(eps128[:], EPS)

        # ---- intermediate DRAM ----
        qkvsend = dram.tile([8, SH * ROWW], bf16)   # dest-major AllToAll stage
        qkv2d = dram.tile([TP, ROWW], bf16)         # [t, (ch, f)] q0-3 k0-3 v0-15
        oint = dram.tile([8, 16 * SHF], bf16)
        oall = dram.tile([128, SHF], bf16)

        x_sh = t["x_sh"]

        # ======== phase 1: QKV proj + PReLU + LN (all heads, both b) ======
        if "1" in phases:
            with tc.tile_pool(name="p1x", bufs=1) as p1x, \
                 tc.tile_pool(name="p1w", bufs=3) as p1w, \
                 tc.tile_pool(name="p1s", bufs=2) as p1s, \
                 tc.tile_pool(name="p1ps", bufs=2, space="PSUM") as p1ps, \
                 tc.tile_pool(name="p1ps1", bufs=1, space="PSUM") as p1ps1:
                xb = []
                for b in range(2):
                    xt = p1x.tile([C, SHF], bf16, name=f"xb{b}")
                    nc.sync.dma_start(xt[:], x_sh[64 * b:64 * b + 64, :])
                    xb.append(xt)
                for it in range(2 * PJ_TILES):
                    b, i = it // PJ_TILES, it % PJ_TILES
                    n0 = i * PJ_N
                    ypsum = p1ps.tile([128, 512], f32, tag="ypsum")
                    nc.tensor.matmul(ypsum[0:96, 0:PJ_N], w96_sb[:],
                                     xb[b][:, n0:n0 + PJ_N],
                                     start=True, stop=True)
                    yp = ypsum[0:96, 0:PJ_N]

                    r1 = p1w.tile([96, PJ_N], f32, tag="r1")
                    nc.scalar.activation(r1[:], yp,
                                         mybir.ActivationFunctionType.Relu,
                                         bias=vecs["bias_v"][:], scale=1.0)
                    r2 = p1w.tile([96, PJ_N], f32, tag="r2")
                    nc.scalar.activation(r2[:], yp,
                                         mybir.ActivationFunctionType.Relu,
                                         bias=vecs["nbias_v"][:], scale=-1.0)
                    r2a = p1w.tile([96, PJ_N], f32, tag="r2a")
                    nc.vector.tensor_scalar(r2a[:], r2[:], vecs["na_v"][:],
                                            None, mybir.AluOpType.mult)
                    y_sb = p1w.tile([96, PJ_N], f32, tag="y_sb")
                    nc.vector.tensor_tensor(y_sb[:], r1[:], r2a[:],
                                            mybir.AluOpType.add)
                    y2 = p1w.tile([96, PJ_N], f32, tag="y2")
                    nc.scalar.activation(y2[:], y_sb[:],
                                         mybir.ActivationFunctionType.Square)

                    mu_psf = p1ps.tile([12, 512], f32, tag="mu_ps")
                    mu_ps = mu_psf[:, 0:PJ_N]
                    nc.tensor.matmul(mu_ps, g_sb[:], y_sb[:],
                                     start=True, stop=True)
                    m2_psf = p1ps.tile([12, 512], f32, tag="m2_ps")
                    m2_ps = m2_psf[:, 0:PJ_N]
                    nc.tensor.matmul(m2_ps, g_sb[:], y2[:],
                                     start=True, stop=True)

                    spair = p1s.tile([12, 2 * PJ_N], f32, tag="spair")
                    nc.scalar.copy(spair[:, 0:PJ_N], mu_ps)
                    musq = p1s.tile([12, PJ_N], f32, tag="musq")
                    nc.vector.tensor_tensor(musq[:], spair[:, 0:PJ_N],
                                            spair[:, 0:PJ_N],
                                            mybir.AluOpType.mult)
                    var = p1s.tile([12, PJ_N], f32, tag="var")
                    nc.vector.tensor_tensor(var[:], m2_ps, musq[:],
                                            mybir.AluOpType.subtract)
                    stdd = p1s.tile([12, PJ_N], f32, tag="stdd")
                    nc.scalar.activation(stdd[:], var[:],
                                         mybir.ActivationFunctionType.Sqrt,
                                         bias=eps128[0:12, :])
                    nc.vector.reciprocal(spair[:, PJ_N:2 * PJ_N], stdd[:])

                    mub = p1ps1.tile([128, 512], f32, tag="mub")
                    nc.tensor.matmul(mub[0:96, 0:PJ_N], bb_sb[:],
                                     spair[:, 0:PJ_N], start=True, stop=True)
                    rsb = p1ps1.tile([128, 512], f32, tag="rsb")
                    nc.tensor.matmul(rsb[0:96, 0:PJ_N], bb_sb[:],
                                     spair[:, PJ_N:2 * PJ_N],
                                     start=True, stop=True)

                    t1 = p1w.tile([96, PJ_N], f32, tag="t1")
                    nc.vector.tensor_tensor(t1[:], y_sb[:], mub[0:96, 0:PJ_N],
                                            mybir.AluOpType.subtract)
                    t2 = p1w.tile([96, PJ_N], f32, tag="t2")
                    nc.vector.tensor_tensor(t2[:], t1[:], rsb[0:96, 0:PJ_N],
                                            mybir.AluOpType.mult)
                    yf = p1w.tile([96, PJ_N], bf16, tag="yf")
                    nc.vector.tensor_scalar(yf[:], t2[:], vecs["gam_v"][:],
                                            vecs["bet_v"][:],
                                            mybir.AluOpType.mult,
                                            mybir.AluOpType.add)

                    # scatter to qkvsend[4b+h, tl*1560 + ch*65 + f]
                    for h in range(4):
                        dst = AP(tensor=qkvsend.tensor,
                                 offset=(4 * b + h) * SH * ROWW
                                 + i * PJ_T * ROWW,
                                 ap=[[F, 24], [ROWW, PJ_T], [1, F]])
                        nc.sync.dma_start(dst, yf[24 * h:24 * h + 24, :])

        # ======== phase C: AllToAll #1 (t-shard -> (b,h)-shard) ==========
        if "C" in phases:
            nc.gpsimd.collective_compute(
                "AllToAll", mybir.AluOpType.bypass,
                replica_groups=[[0, 1, 2, 3, 4, 5, 6, 7]],
                ins=[qkvsend[:]],
                outs=[qkv2d.rearrange("(a t) n -> a (t n)", a=8)],
            )

        # ======== phase 2: load K/Q emb (transpose) + V ==========
        if "2" in phases:
            with tc.tile_pool(name="attp", bufs=1) as attp:
                k_eT = []
                q_eT = []
                for ce, (e0, w) in enumerate(((0, 128), (128, 128), (256, 4))):
                    kt = attp.tile([128, TP], bf16, name=f"k_eT{ce}")
                    qt_ = attp.tile([128, TP], bf16, name=f"q_eT{ce}")
                    for sb in range(NSB):
                        nc.sync.dma_start_transpose(
                            kt[0:w, sb * 512:(sb + 1) * 512],
                            qkv2d[sb * 512:(sb + 1) * 512,
                                  DF + e0:DF + e0 + w])
                        nc.sync.dma_start_transpose(
                            qt_[0:w, sb * 512:(sb + 1) * 512],
                            qkv2d[sb * 512:(sb + 1) * 512, e0:e0 + w])
                    k_eT.append(kt)
                    q_eT.append(qt_)

                v_sb = []
                for st in range(NQT):
                    vt = attp.tile([128, EF], bf16, name=f"v_sb{st}")
                    nc.sync.dma_start(
                        vt[:], qkv2d[st * 128:(st + 1) * 128, 2 * DF:ROWW])
                    v_sb.append(vt)

                # ============== phase 3: attention ==============
                if "3" in phases:
                    with tc.tile_pool(name="a3", bufs=2) as a3, \
                         tc.tile_pool(name="a3p", bufs=7) as a3p, \
                         tc.tile_pool(name="a3ps", bufs=2, space="PSUM") as a3ps, \
                         tc.tile_pool(name="a3po", bufs=1, space="PSUM") as a3po:
                        for qt in range(NQT):
                            qs = slice(qt * 128, (qt + 1) * 128)
                            pblk = []
                            acc6 = a3.tile([128, 8], f32, tag="acc6")
                            for sb in range(NSB):
                                s_ps = a3ps.tile([128, 512], f32, tag="s_ps")
                                for ce, w in ((0, 128), (1, 128), (2, 4)):
                                    nc.tensor.matmul(
                                        s_ps[:], q_eT[ce][0:w, qs],
                                        k_eT[ce][0:w, sb * 512:(sb + 1) * 512],
                                        start=(ce == 0), stop=(ce == 2))
                                pb = a3p.tile([128, 512], bf16, tag=f"pb{sb}")
                                ncols = 512 if sb < NSB - 1 else S_REAL_LAST
                                nc.scalar.activation(
                                    pb[:, 0:ncols], s_ps[:, 0:ncols],
                                    mybir.ActivationFunctionType.Exp,
                                    scale=SCALE, accum_out=acc6[:, sb:sb + 1])
                                if ncols < 512:
                                    nc.vector.memset(pb[:, ncols:512], 0.0)
                                pblk.append(pb)

                            dsum = a3.tile([128, 1], f32, tag="dsum")
                            nc.vector.reduce_sum(dsum[:], acc6[:, 0:NSB],
                                                 axis=mybir.AxisListType.X)
                            rcp = a3.tile([128, 1], f32, tag="rcp")
                            nc.vector.reciprocal(rcp[:], dsum[:])

                            o_ps = a3po.tile([128, 1536], f32, tag="o_ps")
                            for sb in range(NSB):
                                for c4 in range(4):
                                    st = 4 * sb + c4
                                    pt_ps = a3ps.tile([128, 1024], bf16,
                                                      tag="pt_ps")
                                    nc.tensor.transpose(
                                        pt_ps[:, 0:128],
                                        pblk[sb][:, c4 * 128:(c4 + 1) * 128],
                                        ident_sb[:])
                                    pt_sb = a3.tile([128, 128], bf16,
                                                    tag="pt_sb")
                                    nc.vector.tensor_copy(pt_sb[:],
                                                          pt_ps[:, 0:128])
                                    first, last = (st == 0), (st == NQT - 1)
                                    nc.tensor.matmul(o_ps[:, 0:512], pt_sb[:],
                                                     v_sb[st][:, 0:512],
                                                     start=first, stop=last)
                                    nc.tensor.matmul(o_ps[:, 512:1024],
                                                     pt_sb[:],
                                                     v_sb[st][:, 512:1024],
                                                     start=first, stop=last)
                                    nc.tensor.matmul(o_ps[:, 1024:EF],
                                                     pt_sb[:],
                                                     v_sb[st][:, 1024:EF],
                                                     start=first, stop=last)

                            o_sb = a3.tile([128, EF], bf16, tag="o_sb")
                            nc.vector.tensor_scalar(o_sb[:], o_ps[:, 0:EF],
                                                    rcp[:], None,
                                                    mybir.AluOpType.mult)
                            sh, tl0 = qt // 3, (qt % 3) * 128
                            dst = AP(tensor=oint.tensor,
                                     offset=sh * 16 * SHF + tl0 * F,
                                     ap=[[F, 128], [SHF, E], [1, F]])
                            nc.sync.dma_start(dst, o_sb[:])

        # ======== phase A: AllToAll #2 ==========
        if "A" in phases:
            nc.gpsimd.collective_compute(
                "AllToAll", mybir.AluOpType.bypass,
                replica_groups=[[0, 1, 2, 3, 4, 5, 6, 7]],
                ins=[oint[:]],
                outs=[oall.rearrange("(a c) n -> a (c n)", a=8)],
            )

        # ======== phase 5: final proj + LN + residual ==========
        if "5" in phases:
            y_out = t["y_out"]
            with tc.tile_pool(name="p5", bufs=3) as p5, \
                 tc.tile_pool(name="p5ps", bufs=2, space="PSUM") as p5ps:
              nchunks = (SHF + 511) // 512
              for half in range(2):
                for k in range(nchunks):
                    n0 = k * 512
                    n = min(512, SHF - n0)
                    o_c = p5.tile([C, 512], bf16, tag="o_c")
                    nc.sync.dma_start(o_c[:, 0:n],
                                      oall[64 * half:64 * half + 64,
                                           n0:n0 + n])
                    x_c = p5.tile([C, 512], bf16, tag="x_c")
                    nc.sync.dma_start(x_c[:, 0:n],
                                      x_sh[64 * half:64 * half + 64,
                                           n0:n0 + n])

                    y1 = p5ps.tile([C, 512], f32, tag="y1")
                    nc.tensor.matmul(y1[:, 0:n], wpT_sb[:], o_c[:, 0:n],
                                     start=True, stop=True)
                    r1 = p5.tile([C, 512], f32, tag="fr1")
                    nc.scalar.activation(r1[:, 0:n], y1[:, 0:n],
                                         mybir.ActivationFunctionType.Relu,
                                         bias=fvecs["bp_v"][:], scale=1.0)
                    r2 = p5.tile([C, 512], f32, tag="fr2")
                    nc.scalar.activation(r2[:, 0:n], y1[:, 0:n],
                                         mybir.ActivationFunctionType.Relu,
                                         bias=fvecs["nbp_v"][:], scale=-1.0)
                    r2a = p5.tile([C, 512], f32, tag="fr2a")
                    nc.vector.tensor_scalar(r2a[:, 0:n], r2[:, 0:n],
                                            fvecs["nap_v"][:], None,
                                            mybir.AluOpType.mult)
                    s_sb = p5.tile([C, 512], f32, tag="fs")
                    nc.gpsimd.tensor_tensor(s_sb[:, 0:n], r1[:, 0:n],
                                            r2a[:, 0:n], mybir.AluOpType.add)

                    mu = p5ps.tile([C, 512], f32, tag="fmu")
                    nc.tensor.matmul(mu[:, 0:n], ones_sb[:], s_sb[:, 0:n],
                                     start=True, stop=True)
                    t1 = p5.tile([C, 512], f32, tag="ft1")
                    nc.vector.tensor_tensor(t1[:, 0:n], s_sb[:, 0:n],
                                            mu[:, 0:n],
                                            mybir.AluOpType.subtract)
                    sq = p5.tile([C, 512], f32, tag="fsq")
                    nc.scalar.activation(sq[:, 0:n], t1[:, 0:n],
                                         mybir.ActivationFunctionType.Square)
                    vv = p5ps.tile([C, 512], f32, tag="fvar")
                    nc.tensor.matmul(vv[:, 0:n], ones_sb[:], sq[:, 0:n],
                                     start=True, stop=True)
                    stdd = p5.tile([C, 512], f32, tag="fstd")
                    nc.scalar.activation(stdd[:, 0:n], vv[:, 0:n],
                                         mybir.ActivationFunctionType.Sqrt,
                                         bias=eps128[0:C, :])
                    rstd = p5.tile([C, 512], f32, tag="frstd")
                    nc.vector.reciprocal(rstd[:, 0:n], stdd[:, 0:n])
                    yn = p5.tile([C, 512], f32, tag="fyn")
                    nc.vector.tensor_tensor(yn[:, 0:n], t1[:, 0:n],
                                            rstd[:, 0:n],
                                            mybir.AluOpType.mult)
                    yg = p5.tile([C, 512], f32, tag="fyg")
                    nc.vector.tensor_scalar(yg[:, 0:n], yn[:, 0:n],
                                            fvecs["gp_v"][:],
                                            fvecs["betp_v"][:],
                                            mybir.AluOpType.mult,
                                            mybir.AluOpType.add)
                    yo = p5.tile([C, 512], bf16, tag="fyo")
                    nc.gpsimd.tensor_tensor(yo[:, 0:n], yg[:, 0:n],
                                            x_c[:, 0:n], mybir.AluOpType.add)
                    nc.sync.dma_start(y_out[64 * half:64 * half + 64,
                                            n0:n0 + n], yo[:, 0:n])


_PROGRAM = None


def _get_program():
    global _PROGRAM
    if _PROGRAM is None:
        _PROGRAM = _build_program()
    return _PROGRAM


def _core_inputs(inp, c):
    x = np.asarray(inp["x"], np.float32)
    xb = np.zeros((B, C, TP, F), np.float32)
    xb[:, :, :T, :] = x
    # core c's t-shard of BOTH batches, [2C, SHF], bf16
    x_sh = np.ascontiguousarray(
        xb[:, :, SH * c:SH * (c + 1), :].reshape(2 * C, SHF)).astype(BF16)

    Wq, Wk, Wv = (np.asarray(inp[k], np.float32) for k in ("Wq", "Wk", "Wv"))
    bq, bk, bv = (np.asarray(inp[k], np.float32) for k in ("bq", "bk", "bv"))
    aq, ak, av = (np.asarray(inp[k], np.float32) for k in ("aq", "ak", "av"))
    gq, gk, gv = (np.asarray(inp[k], np.float32) for k in ("gq", "gk", "gv"))
    btq, btk, btv = (np.asarray(inp[k], np.float32)
                     for k in ("betaq", "betak", "betav"))

    w96 = np.zeros((C, 96), np.float32)
    bias_v = np.zeros((96, 1), np.float32)
    na_v = np.zeros((96, 1), np.float32)
    gam_v = np.zeros((96, 1), np.float32)
    bet_v = np.zeros((96, 1), np.float32)
    G = np.zeros((96, 12), np.float32)
    Bbm = np.zeros((12, 96), np.float32)
    for h in range(4):
        r = 24 * h
        w24 = np.concatenate([Wq[h], Wk[h], Wv[h]], axis=0)     # [24, C]
        w96[:, r:r + 24] = w24.T
        bias_v[r:r + 24, 0] = np.concatenate([bq[h], bk[h], bv[h]])
        a24 = np.concatenate([np.full(D, aq[h]), np.full(D, ak[h]),
                              np.full(E, av[h])]).astype(np.float32)
        na_v[r:r + 24, 0] = -a24
        gam_v[r:r + 24, 0] = np.concatenate([gq[h], gk[h], gv[h]])
        bet_v[r:r + 24, 0] = np.concatenate([btq[h], btk[h], btv[h]])
        G[r:r + 4, 3 * h + 0] = 0.25
        G[r + 4:r + 8, 3 * h + 1] = 0.25
        G[r + 8:r + 24, 3 * h + 2] = 1.0 / 16.0
        Bbm[3 * h + 0, r:r + 4] = 1.0
        Bbm[3 * h + 1, r + 4:r + 8] = 1.0
        Bbm[3 * h + 2, r + 8:r + 24] = 1.0

    Wp = np.asarray(inp["Wp"], np.float32)
    bp = np.asarray(inp["bp"], np.float32)
    ap = np.float32(inp["ap"])
    gp = np.asarray(inp["gp"], np.float32)
    betp = np.asarray(inp["betap"], np.float32)

    return {
        "x_sh": x_sh,
        "w96": w96.astype(BF16),
        "bias_v": bias_v,
        "nbias_v": -bias_v,
        "na_v": na_v,
        "gam_v": gam_v,
        "bet_v": bet_v,
        "Gm": G,
        "Bb": Bbm,
        "wpT": np.ascontiguousarray(Wp.T).astype(BF16),
        "ones64": np.full((C, C), 1.0 / 64.0, np.float32),
        "nap_v": np.full((C, 1), -ap, np.float32),
        "bp_v": bp.reshape(C, 1).copy(),
        "nbp_v": (-bp).reshape(C, 1).copy(),
        "gp_v": gp.reshape(C, 1).copy(),
        "betp_v": betp.reshape(C, 1).copy(),
        "ident": np.eye(128, dtype=BF16),
    }


def gather_output(results):
    y = np.empty((B, C, T, F), np.float32)
    for c in range(8):
        sh = np.asarray(results[c]["y_shard"], np.float32).reshape(B, C, SH, F)
        t0, t1 = SH * c, min(SH * (c + 1), T)
        if t1 > t0:
            y[:, :, t0:t1, :] = sh[:, :, :t1 - t0, :]
    return y


def kernel(**inputs):
    nc = _get_program()
    in_maps = [_core_inputs(inputs, c) for c in range(8)]
    res = run_bass_kernel_spmd(nc, in_maps, core_ids=list(range(8)))
    return gather_output(res.results)


# revision 9
# speedup vs baseline: 8.1746x; 1.0288x over previous
"""MultiHeadSelfAttention2D Trainium2 kernel (8-core SPMD).

v2: input-minimal T-sharded design.

Each core receives ONLY its 1/8 time-shard of x (both batches, bf16,
~6.4 MB) plus small weight tensors.  Flow per core:

  phase 1: QKV 1x1-conv + PReLU + channel-LN for ALL 4 heads and BOTH
           batches on the core's 384-t shard (96 output channels).
  phase C: AllToAll #1 redistributes QKV t-shards -> (batch, head)
           shards; core d = 4*b + h ends with full-T qkv2d [3072, 1560]
           in [t, (ch, f)] layout for its (b, h).
  phase 2: load K/Q embeddings (DMA transpose) + V into SBUF.
  phase 3: full attention over T (exp without max-subtraction --
           LN-bounded scores), P^T V accumulation.
  phase A: AllToAll #2 exchanges per-head attention outputs so each
           core holds all 16 v-chan groups for a 384-t shard of both
           batches.
  phase 5: final concat 1x1-conv + PReLU + channel-LN + residual on the
           same t-shard (residual re-reads x_sh).

All shapes hardcoded for the problem instance:
  x [2, 64, 3000, 65], H=4 heads, D=4 q/k chans, E=16 v chans.
"""

import numpy as np
import ml_dtypes

import concourse.bass as bass
import concourse.mybir as mybir
import concourse.tile as tile
from concourse import bacc
from concourse.bass_utils import run_bass_kernel_spmd

BF16 = ml_dtypes.bfloat16

B, C, T, F = 2, 64, 3000, 65
H, D, E = 4, 4, 16
TP = 3072                    # padded T (24 tiles of 128)
DF = D * F                   # 260  q/k embedding
EF = E * F                   # 1040 v embedding
SH = TP // 8                 # 384  t-shard per core per batch
SHF = SH * F                 # 24960
ROWW = 24 * F                # 1560 qkv2d row width: 24 chans x 65 f
SCALE = float(1.0 / np.sqrt(np.float32(DF)))
EPS = 1e-5

f32 = mybir.dt.float32
bf16 = mybir.dt.bfloat16

# phase-1 tiling: chunk of 6 t (390 (t,f) positions) per iteration
PJ_T = 6
PJ_N = PJ_T * F              # 390 free
PJ_TILES = SH // PJ_T        # 64 chunks per batch

NQT = TP // 128              # 24 q tiles
NSB = TP // 512              # 6 s blocks of 512
S_REAL_LAST = T - 5 * 512    # 440 real cols in s-block 5


def _build_program(nrep=1, phases="1C23A5"):
    nc = bacc.Bacc("TRN2", target_bir_lowering=False, debug=False,
                   num_devices=8)

    def din(name, shape, dt=f32):
        return nc.dram_tensor(name, list(shape), dt, kind="ExternalInput")

    x_sh = din("x_sh", [2 * C, SHF], bf16)
    w96 = din("w96", [C, 96], bf16)
    bias_v = din("bias_v", [96, 1])
    nbias_v = din("nbias_v", [96, 1])
    na_v = din("na_v", [96, 1])
    gam_v = din("gam_v", [96, 1])
    bet_v = din("bet_v", [96, 1])
    Gm = din("Gm", [96, 12], bf16)
    Bb = din("Bb", [12, 96], bf16)
    wpT = din("wpT", [C, C], bf16)
    ones64 = din("ones64", [C, C], bf16)
    nap_v = din("nap_v", [C, 1])
    bp_v = din("bp_v", [C, 1])
    nbp_v = din("nbp_v", [C, 1])
    gp_v = din("gp_v", [C, 1])
    betp_v = din("betp_v", [C, 1])
    ident_in = din("ident", [128, 128], bf16)

    y_out = nc.dram_tensor("y_shard", [2 * C, SHF], bf16, kind="ExternalOutput")

    env = locals()
    with tile.TileContext(nc) as tc, \
         nc.allow_low_precision(reason="bf16 LN stats; rel-err budget 2e-2"):
        for _rep in range(nrep):
            _body(tc, env, phases)
    nc.compile()
    return nc


def _body(tc, t, phases="1C23A5"):
    nc = tc.nc
    AP = bass.AP

    with tc.tile_pool(name="consts", bufs=1) as consts, \
         tc.tile_pool(name="dram", bufs=1, space="DRAM") as dram:

        # ---- constants into SBUF ----
        w96_sb = consts.tile([C, 96], bf16)
        nc.sync.dma_start(w96_sb[:], t["w96"][:])
        g_sb = consts.tile([96, 12], bf16)
        nc.sync.dma_start(g_sb[:], t["Gm"][:])
        bb_sb = consts.tile([12, 96], bf16)
        nc.sync.dma_start(bb_sb[:], t["Bb"][:])
        vecs = {}
        for nm in ("bias_v", "nbias_v", "na_v", "gam_v", "bet_v"):
            v = consts.tile([96, 1], f32, name=nm + "_sb")
            nc.sync.dma_start(v[:], t[nm][:])
            vecs[nm] = v
        fvecs = {}
        for nm in ("nap_v", "bp_v", "nbp_v", "gp_v", "betp_v"):
            v = consts.tile([C, 1], f32, name=nm + "_sb")
            nc.sync.dma_start(v[:], t[nm][:])
            fvecs[nm] = v
        wpT_sb = consts.tile([C, C], bf16)
        nc.sync.dma_start(wpT_sb[:], t["wpT"][:])
        ones_sb = consts.tile([C, C], bf16)
        nc.sync.dma_start(ones_sb[:], t["ones64"][:])
        ident_sb = consts.tile([128, 128], bf16)
        nc.sync.dma_start(ident_sb[:], t["ident_in"][:])
        eps128 = consts.tile([128, 1], f32)
        nc.vector.memset(eps128[:], EPS)

        # ---- intermediate DRAM ----
        qkvsend = dram.tile([8, SH * ROWW], bf16)   # dest-major AllToAll stage
        qkv2d = dram.tile([TP, ROWW], bf16)         # [t, (ch, f)] q0-3 k0-3 v0-15
        oint = dram.tile([8, 16 * SHF], bf16)
        oall = dram.tile([128, SHF], bf16)

        x_sh = t["x_sh"]

        # ======== phase 1: QKV proj + PReLU + LN (all heads, both b) ======
        if "1" in phases:
            with tc.tile_pool(name="p1x", bufs=1) as p1x, \
                 tc.tile_pool(name="p1w", bufs=3) as p1w, \
                 tc.tile_pool(name="p1s", bufs=2) as p1s, \
                 tc.tile_pool(name="p1ps", bufs=2, space="PSUM") as p1ps, \
                 tc.tile_pool(name="p1ps1", bufs=1, space="PSUM") as p1ps1:
                xb = []
                for b in range(2):
                    xt = p1x.tile([C, SHF], bf16, name=f"xb{b}")
                    nc.sync.dma_start(xt[:], x_sh[64 * b:64 * b + 64, :])
                    xb.append(xt)
                for it in range(2 * PJ_TILES):
                    b, i = it // PJ_TILES, it % PJ_TILES
                    n0 = i * PJ_N
                    ypsum = p1ps.tile([128, 512], f32, tag="ypsum")
                    nc.tensor.matmul(ypsum[0:96, 0:PJ_N], w96_sb[:],
                                     xb[b][:, n0:n0 + PJ_N],
                                     start=True, stop=True)
                    yp = ypsum[0:96, 0:PJ_N]

                    r1 = p1w.tile([96, PJ_N], bf16, tag="r1")
                    nc.scalar.activation(r1[:], yp,
                                         mybir.ActivationFunctionType.Relu,
                                         bias=vecs["bias_v"][:], scale=1.0)
                    r2 = p1w.tile([96, PJ_N], bf16, tag="r2")
                    nc.scalar.activation(r2[:], yp,
                                         mybir.ActivationFunctionType.Relu,
                                         bias=vecs["nbias_v"][:], scale=-1.0)
                    r2a = p1w.tile([96, PJ_N], bf16, tag="r2a")
                    nc.vector.tensor_scalar(r2a[:], r2[:], vecs["na_v"][:],
                                            None, mybir.AluOpType.mult)
                    y_sb = p1w.tile([96, PJ_N], bf16, tag="y_sb")
                    nc.gpsimd.tensor_tensor(y_sb[:], r1[:], r2a[:],
                                            mybir.AluOpType.add)

                    # two-pass LN stats: var = E[(y-mu)^2] -- no
                    # cancellation, safe with bf16 operands
                    mu_psf = p1ps.tile([12, 512], f32, tag="mu_ps")
                    mu_ps = mu_psf[:, 0:PJ_N]
                    nc.tensor.matmul(mu_ps, g_sb[:], y_sb[:],
                                     start=True, stop=True)
                    mu_sb = p1s.tile([12, PJ_N], bf16, tag="mu_sb")
                    nc.scalar.copy(mu_sb[:], mu_ps)
                    mub = p1ps1.tile([128, 512], f32, tag="mub")
                    nc.tensor.matmul(mub[0:96, 0:PJ_N], bb_sb[:],
                                     mu_sb[:], start=True, stop=True)
                    t1 = p1w.tile([96, PJ_N], bf16, tag="t1")
                    nc.vector.tensor_tensor(t1[:], y_sb[:], mub[0:96, 0:PJ_N],
                                            mybir.AluOpType.subtract)
                    t1sq = p1w.tile([96, PJ_N], bf16, tag="t1sq")
                    nc.scalar.activation(t1sq[:], t1[:],
                                         mybir.ActivationFunctionType.Square)
                    v_psf = p1ps.tile([12, 512], f32, tag="v_ps")
                    v_ps = v_psf[:, 0:PJ_N]
                    nc.tensor.matmul(v_ps, g_sb[:], t1sq[:],
                                     start=True, stop=True)
                    stdd = p1s.tile([12, PJ_N], f32, tag="stdd")
                    nc.scalar.activation(stdd[:], v_ps,
                                         mybir.ActivationFunctionType.Sqrt,
                                         bias=eps128[0:12, :])
                    rcp12 = p1s.tile([12, PJ_N], bf16, tag="rcp12")
                    nc.vector.reciprocal(rcp12[:], stdd[:])
                    rsb = p1ps1.tile([128, 512], f32, tag="rsb")
                    nc.tensor.matmul(rsb[0:96, 0:PJ_N], bb_sb[:],
                                     rcp12[:], start=True, stop=True)
                    t2 = p1w.tile([96, PJ_N], bf16, tag="t2")
                    nc.vector.tensor_tensor(t2[:], t1[:], rsb[0:96, 0:PJ_N],
                                            mybir.AluOpType.mult)
                    yf = p1w.tile([96, PJ_N], bf16, tag="yf")
                    nc.vector.tensor_scalar(yf[:], t2[:], vecs["gam_v"][:],
                                            vecs["bet_v"][:],
                                            mybir.AluOpType.mult,
                                            mybir.AluOpType.add)

                    # scatter to qkvsend[4b+h, tl*1560 + ch*65 + f]
                    for h in range(4):
                        dst = AP(tensor=qkvsend.tensor,
                                 offset=(4 * b + h) * SH * ROWW
                                 + i * PJ_T * ROWW,
                                 ap=[[F, 24], [ROWW, PJ_T], [1, F]])
                        nc.sync.dma_start(dst, yf[24 * h:24 * h + 24, :])

        # ======== phase C: AllToAll #1 (t-shard -> (b,h)-shard) ==========
        if "C" in phases:
            nc.gpsimd.collective_compute(
                "AllToAll", mybir.AluOpType.bypass,
                replica_groups=[[0, 1, 2, 3, 4, 5, 6, 7]],
                ins=[qkvsend[:]],
                outs=[qkv2d.rearrange("(a t) n -> a (t n)", a=8)],
            )

        # ======== phase 2: load K/Q emb (transpose) + V ==========
        if "2" in phases:
            with tc.tile_pool(name="attp", bufs=1) as attp:
                k_eT = []
                q_eT = []
                for ce, (e0, w) in enumerate(((0, 128), (128, 128), (256, 4))):
                    kt = attp.tile([128, TP], bf16, name=f"k_eT{ce}")
                    qt_ = attp.tile([128, TP], bf16, name=f"q_eT{ce}")
                    for sb in range(NSB):
                        nc.sync.dma_start_transpose(
                            kt[0:w, sb * 512:(sb + 1) * 512],
                            qkv2d[sb * 512:(sb + 1) * 512,
                                  DF + e0:DF + e0 + w])
                        nc.sync.dma_start_transpose(
                            qt_[0:w, sb * 512:(sb + 1) * 512],
                            qkv2d[sb * 512:(sb + 1) * 512, e0:e0 + w])
                    k_eT.append(kt)
                    q_eT.append(qt_)

                v_sb = []
                for st in range(NQT):
                    vt = attp.tile([128, EF], bf16, name=f"v_sb{st}")
                    nc.sync.dma_start(
                        vt[:], qkv2d[st * 128:(st + 1) * 128, 2 * DF:ROWW])
                    v_sb.append(vt)

                # ============== phase 3: attention ==============
                if "3" in phases:
                    with tc.tile_pool(name="a3", bufs=2) as a3, \
                         tc.tile_pool(name="a3p", bufs=7) as a3p, \
                         tc.tile_pool(name="a3ps", bufs=2, space="PSUM") as a3ps, \
                         tc.tile_pool(name="a3po", bufs=1, space="PSUM") as a3po:
                        for qt in range(NQT):
                            qs = slice(qt * 128, (qt + 1) * 128)
                            pblk = []
                            acc6 = a3.tile([128, 8], f32, tag="acc6")
                            for sb in range(NSB):
                                s_ps = a3ps.tile([128, 512], f32, tag="s_ps")
                                for ce, w in ((0, 128), (1, 128), (2, 4)):
                                    nc.tensor.matmul(
                                        s_ps[:], q_eT[ce][0:w, qs],
                                        k_eT[ce][0:w, sb * 512:(sb + 1) * 512],
                                        start=(ce == 0), stop=(ce == 2))
                                pb = a3p.tile([128, 512], bf16, tag=f"pb{sb}")
                                ncols = 512 if sb < NSB - 1 else S_REAL_LAST
                                nc.scalar.activation(
                                    pb[:, 0:ncols], s_ps[:, 0:ncols],
                                    mybir.ActivationFunctionType.Exp,
                                    scale=SCALE, accum_out=acc6[:, sb:sb + 1])
                                if ncols < 512:
                                    nc.vector.memset(pb[:, ncols:512], 0.0)
                                pblk.append(pb)

                            dsum = a3.tile([128, 1], f32, tag="dsum")
                            nc.vector.reduce_sum(dsum[:], acc6[:, 0:NSB],
                                                 axis=mybir.AxisListType.X)
                            rcp = a3.tile([128, 1], f32, tag="rcp")
                            nc.vector.reciprocal(rcp[:], dsum[:])

                            o_ps = a3po.tile([128, 1536], f32, tag="o_ps")
                            for sb in range(NSB):
                                for c4 in range(4):
                                    st = 4 * sb + c4
                                    pt_ps = a3ps.tile([128, 1024], bf16,
                                                      tag="pt_ps")
                                    nc.tensor.transpose(
                                        pt_ps[:, 0:128],
                                        pblk[sb][:, c4 * 128:(c4 + 1) * 128],
                                        ident_sb[:])
                                    pt_sb = a3.tile([128, 128], bf16,
                                                    tag="pt_sb")
                                    nc.vector.tensor_copy(pt_sb[:],
                                                          pt_ps[:, 0:128])
                                    first, last = (st == 0), (st == NQT - 1)
                                    nc.tensor.matmul(o_ps[:, 0:512], pt_sb[:],
                                                     v_sb[st][:, 0:512],
                                                     start=first, stop=last)
                                    nc.tensor.matmul(o_ps[:, 512:1024],
                                                     pt_sb[:],
                                                     v_sb[st][:, 512:1024],
                                                     start=first, stop=last)
                                    nc.tensor.matmul(o_ps[:, 1024:EF],
                                                     pt_sb[:],
                                                     v_sb[st][:, 1024:EF],
                                                     start=first, stop=last)

                            o_sb = a3.tile([128, EF], bf16, tag="o_sb")
                            nc.vector.tensor_scalar(o_sb[:], o_ps[:, 0:EF],
                                                    rcp[:], None,
                                                    mybir.AluOpType.mult)
                            sh, tl0 = qt // 3, (qt % 3) * 128
                            dst = AP(tensor=oint.tensor,
                                     offset=sh * 16 * SHF + tl0 * F,
                                     ap=[[F, 128], [SHF, E], [1, F]])
                            nc.sync.dma_start(dst, o_sb[:])

        # ======== phase A: AllToAll #2 ==========
        if "A" in phases:
            nc.gpsimd.collective_compute(
                "AllToAll", mybir.AluOpType.bypass,
                replica_groups=[[0, 1, 2, 3, 4, 5, 6, 7]],
                ins=[oint[:]],
                outs=[oall.rearrange("(a c) n -> a (c n)", a=8)],
            )

        # ======== phase 5: final proj + LN + residual ==========
        if "5" in phases:
            y_out = t["y_out"]
            with tc.tile_pool(name="p5", bufs=3) as p5, \
                 tc.tile_pool(name="p5ps", bufs=2, space="PSUM") as p5ps:
              nchunks = (SHF + 511) // 512
              for half in range(2):
                for k in range(nchunks):
                    n0 = k * 512
                    n = min(512, SHF - n0)
                    o_c = p5.tile([C, 512], bf16, tag="o_c")
                    nc.sync.dma_start(o_c[:, 0:n],
                                      oall[64 * half:64 * half + 64,
                                           n0:n0 + n])
                    x_c = p5.tile([C, 512], bf16, tag="x_c")
                    nc.sync.dma_start(x_c[:, 0:n],
                                      x_sh[64 * half:64 * half + 64,
                                           n0:n0 + n])

                    y1 = p5ps.tile([C, 512], f32, tag="y1")
                    nc.tensor.matmul(y1[:, 0:n], wpT_sb[:], o_c[:, 0:n],
                                     start=True, stop=True)
                    r1 = p5.tile([C, 512], bf16, tag="fr1")
                    nc.scalar.activation(r1[:, 0:n], y1[:, 0:n],
                                         mybir.ActivationFunctionType.Relu,
                                         bias=fvecs["bp_v"][:], scale=1.0)
                    r2 = p5.tile([C, 512], bf16, tag="fr2")
                    nc.scalar.activation(r2[:, 0:n], y1[:, 0:n],
                                         mybir.ActivationFunctionType.Relu,
                                         bias=fvecs["nbp_v"][:], scale=-1.0)
                    r2a = p5.tile([C, 512], bf16, tag="fr2a")
                    nc.vector.tensor_scalar(r2a[:, 0:n], r2[:, 0:n],
                                            fvecs["nap_v"][:], None,
                                            mybir.AluOpType.mult)
                    s_sb = p5.tile([C, 512], bf16, tag="fs")
                    nc.gpsimd.tensor_tensor(s_sb[:, 0:n], r1[:, 0:n],
                                            r2a[:, 0:n], mybir.AluOpType.add)

                    mu = p5ps.tile([C, 512], f32, tag="fmu")
                    nc.tensor.matmul(mu[:, 0:n], ones_sb[:], s_sb[:, 0:n],
                                     start=True, stop=True)
                    t1 = p5.tile([C, 512], f32, tag="ft1")
                    nc.vector.tensor_tensor(t1[:, 0:n], s_sb[:, 0:n],
                                            mu[:, 0:n],
                                            mybir.AluOpType.subtract)
                    sq = p5.tile([C, 512], bf16, tag="fsq")
                    nc.scalar.activation(sq[:, 0:n], t1[:, 0:n],
                                         mybir.ActivationFunctionType.Square)
                    vv = p5ps.tile([C, 512], f32, tag="fvar")
                    nc.tensor.matmul(vv[:, 0:n], ones_sb[:], sq[:, 0:n],
                                     start=True, stop=True)
                    stdd = p5.tile([C, 512], f32, tag="fstd")
                    nc.scalar.activation(stdd[:, 0:n], vv[:, 0:n],
                                         mybir.ActivationFunctionType.Sqrt,
                                         bias=eps128[0:C, :])
                    rstd = p5.tile([C, 512], f32, tag="frstd")
                    nc.vector.reciprocal(rstd[:, 0:n], stdd[:, 0:n])
                    yn = p5.tile([C, 512], f32, tag="fyn")
                    nc.vector.tensor_tensor(yn[:, 0:n], t1[:, 0:n],
                                            rstd[:, 0:n],
                                            mybir.AluOpType.mult)
                    yg = p5.tile([C, 512], f32, tag="fyg")
                    nc.vector.tensor_scalar(yg[:, 0:n], yn[:, 0:n],
                                            fvecs["gp_v"][:],
                                            fvecs["betp_v"][:],
                                            mybir.AluOpType.mult,
                                            mybir.AluOpType.add)
                    yo = p5.tile([C, 512], bf16, tag="fyo")
                    nc.gpsimd.tensor_tensor(yo[:, 0:n], yg[:, 0:n],
                                            x_c[:, 0:n], mybir.AluOpType.add)
                    nc.sync.dma_start(y_out[64 * half:64 * half + 64,
                                            n0:n0 + n], yo[:, 0:n])


_PROGRAM = None


def _get_program():
    global _PROGRAM
    if _PROGRAM is None:
        _PROGRAM = _build_program()
    return _PROGRAM


def _core_inputs(inp, c):
    x = np.asarray(inp["x"], np.float32)
    xb = np.zeros((B, C, TP, F), np.float32)
    xb[:, :, :T, :] = x
    # core c's t-shard of BOTH batches, [2C, SHF], bf16
    x_sh = np.ascontiguousarray(
        xb[:, :, SH * c:SH * (c + 1), :].reshape(2 * C, SHF)).astype(BF16)

    Wq, Wk, Wv = (np.asarray(inp[k], np.float32) for k in ("Wq", "Wk", "Wv"))
    bq, bk, bv = (np.asarray(inp[k], np.float32) for k in ("bq", "bk", "bv"))
    aq, ak, av = (np.asarray(inp[k], np.float32) for k in ("aq", "ak", "av"))
    gq, gk, gv = (np.asarray(inp[k], np.float32) for k in ("gq", "gk", "gv"))
    btq, btk, btv = (np.asarray(inp[k], np.float32)
                     for k in ("betaq", "betak", "betav"))

    w96 = np.zeros((C, 96), np.float32)
    bias_v = np.zeros((96, 1), np.float32)
    na_v = np.zeros((96, 1), np.float32)
    gam_v = np.zeros((96, 1), np.float32)
    bet_v = np.zeros((96, 1), np.float32)
    G = np.zeros((96, 12), np.float32)
    Bbm = np.zeros((12, 96), np.float32)
    for h in range(4):
        r = 24 * h
        w24 = np.concatenate([Wq[h], Wk[h], Wv[h]], axis=0)     # [24, C]
        w96[:, r:r + 24] = w24.T
        bias_v[r:r + 24, 0] = np.concatenate([bq[h], bk[h], bv[h]])
        a24 = np.concatenate([np.full(D, aq[h]), np.full(D, ak[h]),
                              np.full(E, av[h])]).astype(np.float32)
        na_v[r:r + 24, 0] = -a24
        gam_v[r:r + 24, 0] = np.concatenate([gq[h], gk[h], gv[h]])
        bet_v[r:r + 24, 0] = np.concatenate([btq[h], btk[h], btv[h]])
        G[r:r + 4, 3 * h + 0] = 0.25
        G[r + 4:r + 8, 3 * h + 1] = 0.25
        G[r + 8:r + 24, 3 * h + 2] = 1.0 / 16.0
        Bbm[3 * h + 0, r:r + 4] = 1.0
        Bbm[3 * h + 1, r + 4:r + 8] = 1.0
        Bbm[3 * h + 2, r + 8:r + 24] = 1.0

    Wp = np.asarray(inp["Wp"], np.float32)
    bp = np.asarray(inp["bp"], np.float32)
    ap = np.float32(inp["ap"])
    gp = np.asarray(inp["gp"], np.float32)
    betp = np.asarray(inp["betap"], np.float32)

    return {
        "x_sh": x_sh,
        "w96": w96.astype(BF16),
        "bias_v": bias_v,
        "nbias_v": -bias_v,
        "na_v": na_v,
        "gam_v": gam_v,
        "bet_v": bet_v,
        "Gm": G.astype(BF16),
        "Bb": Bbm.astype(BF16),
        "wpT": np.ascontiguousarray(Wp.T).astype(BF16),
        "ones64": np.full((C, C), 1.0 / 64.0, BF16),
        "nap_v": np.full((C, 1), -ap, np.float32),
        "bp_v": bp.reshape(C, 1).copy(),
        "nbp_v": (-bp).reshape(C, 1).copy(),
        "gp_v": gp.reshape(C, 1).copy(),
        "betp_v": betp.reshape(C, 1).copy(),
        "ident": np.eye(128, dtype=BF16),
    }


def gather_output(results):
    y = np.empty((B, C, T, F), np.float32)
    for c in range(8):
        sh = np.asarray(results[c]["y_shard"], np.float32).reshape(B, C, SH, F)
        t0, t1 = SH * c, min(SH * (c + 1), T)
        if t1 > t0:
            y[:, :, t0:t1, :] = sh[:, :, :t1 - t0, :]
    return y


def kernel(**inputs):
    nc = _get_program()
    in_maps = [_core_inputs(inputs, c) for c in range(8)]
    res = run_bass_kernel_spmd(nc, in_maps, core_ids=list(range(8)))
    return gather_output(res.results)
